# revision 1
# baseline (speedup 1.0000x reference)
"""Distributed GPT-2 attention block for 8 TRN2 NeuronCores.

Sharding: core i handles heads {2i, 2i+1} for BOTH batches (tensor-parallel
column split of c_attn). After attention, one 8-core AllToAll converts
head-sharding to token-sharding (512-token slice of the flattened [4096]
token axis per core), then each core runs c_proj (full 1024-feature
contraction) for its slice. Host unshard is pure concatenation.

Layout notes: hidden_states is passed pre-transposed [NX, B*S] (bf16, the
kernel's compute dtype) so q,k stay in [d, t] layout; scores are computed
as S^T = kT.T @ qT; the softmax denominator comes for free from an appended
ones-column in v during the PV matmul; causal masking = block skipping +
affine_select zeroing on diagonal tiles; exp and the 1/sqrt(d) scale are
fused into one ACT pass. All matmuls accumulate in f32 PSUM.
"""

import numpy as np
import ml_dtypes
from contextlib import ExitStack

import concourse.bass as bass
import concourse.bacc as bacc
import concourse.mybir as mybir
import concourse.tile as tile
from concourse.bass_utils import run_bass_kernel_spmd

B, S, NX = 2, 2048, 1024
H, D = 16, 64
HPC = 2              # heads per core
GF = HPC * D         # 128 features per head group
NCORES = 8
SF = B * S           # 4096 flattened tokens
TSL = SF // NCORES   # 512-token output slice per core

F32 = mybir.dt.float32
BF16 = mybir.dt.bfloat16


def build(zero_attn_bias: bool, zero_proj_bias: bool) -> bass.Bass:
    nc = bacc.Bacc(None)

    hst = nc.declare_dram_parameter("hst", [NX, SF], BF16, isOutput=False)
    wqkv = nc.declare_dram_parameter("wqkv", [NX, 3 * GF], BF16, isOutput=False)
    bqkv = nc.declare_dram_parameter("bqkv", [3 * GF, 1], F32, isOutput=False)
    wproj = nc.declare_dram_parameter("wproj", [NX, NX], BF16, isOutput=False)
    bproj = nc.declare_dram_parameter("bproj", [NX, 1], F32, isOutput=False)
    out_ext = nc.declare_dram_parameter("out", [NX, TSL], F32, isOutput=True)

    KT = NX // 128   # 8 k tiles

    with tile.TileContext(nc) as tc, ExitStack() as ctx:
        pool1 = ctx.enter_context(tc.tile_pool(name="persist", bufs=1))
        small = ctx.enter_context(tc.tile_pool(name="small", bufs=2))
        ppool = ctx.enter_context(tc.tile_pool(name="ppool", bufs=2))
        psum = ctx.enter_context(tc.tile_pool(name="psum", bufs=2, space="PSUM"))
        psum_av = ctx.enter_context(tc.tile_pool(name="psum_av", bufs=2, space="PSUM"))
        psum_rb = ctx.enter_context(tc.tile_pool(name="psum_rb", bufs=2, space="PSUM"))
        dram = ctx.enter_context(tc.tile_pool(name="dram", bufs=1, space="DRAM"))

        # ---- load weights and hidden states (bf16, direct, no staging) ------
        # 3D-AP DMAs: [part, ktile, col] <- DRAM[ktile*128 + part, col]
        wqkv_bf = pool1.tile([128, KT, 3 * GF], BF16)
        nc.sync.dma_start(
            wqkv_bf[:], wqkv[:, :].rearrange("(kt p) c -> p kt c", p=128))
        hst_bf = pool1.tile([128, KT, SF], BF16)
        for q in range(4):
            tsl = slice(q * (SF // 4), (q + 1) * (SF // 4))
            for kt in range(KT):
                nc.sync.dma_start(
                    hst_bf[:, kt, tsl], hst[kt * 128:(kt + 1) * 128, tsl])

        # biases as per-partition tiles (q/k: feature-per-partition in qkT layout)
        bqk_t = pool1.tile([128, 2, 1], F32)   # ft 0 = q(2 heads), ft 1 = k
        if not zero_attn_bias:
            for ft in range(2):
                nc.sync.dma_start(bqk_t[:, ft, :], bqkv[ft * 128:(ft + 1) * 128, :])
        bv_t = pool1.tile([64, HPC, 1], F32)
        if not zero_attn_bias:
            for h in range(HPC):
                nc.sync.dma_start(
                    bv_t[:, h, :], bqkv[2 * GF + h * D:2 * GF + (h + 1) * D, :])
        bproj_t = pool1.tile([128, KT, 1], F32)
        if not zero_proj_bias:
            nc.sync.dma_start(
                bproj_t[:], bproj[:, :].rearrange("(kt p) c -> p kt c", p=128))

        # ---- QKV projection --------------------------------------------------
        # q,k transposed: qk_sb[:, ft, t]; ft 0 = q (2 heads), ft 1 = k
        qk_sb = pool1.tile([128, 2, SF], BF16)
        v_sb = pool1.tile([128, SF // 128, HPC, D + 1], BF16)

        def project_batch(b):
            for ft in range(2):
                for tch in range(4 * b, 4 * b + 4):
                    ps = psum.tile([128, 2, 512], F32, tag="mm")
                    for kt in range(KT):
                        nc.tensor.matmul(
                            ps[:, 0, :],
                            lhsT=wqkv_bf[:, kt, ft * 128:(ft + 1) * 128],
                            rhs=hst_bf[:, kt, tch * 512:(tch + 1) * 512],
                            start=(kt == 0), stop=(kt == KT - 1),
                        )
                    if zero_attn_bias:
                        nc.vector.tensor_copy(
                            qk_sb[:, ft, tch * 512:(tch + 1) * 512], ps[:, 0, :])
                    else:
                        nc.scalar.activation(
                            qk_sb[:, ft, tch * 512:(tch + 1) * 512], ps[:, 0, :],
                            mybir.ActivationFunctionType.Identity,
                            bias=bqk_t[:, ft, :])
            # v natural [token, feat] + ones column: v_sb [128, tt, h, 65]
            for tt in range(16 * b, 16 * b + 16):
                ps = psum.tile([128, 2, 512], F32, tag="mm")
                for kt in range(KT):
                    nc.tensor.matmul(
                        ps[:, 0, 0:GF],
                        lhsT=hst_bf[:, kt, tt * 128:(tt + 1) * 128],
                        rhs=wqkv_bf[:, kt, 2 * GF:3 * GF],
                        start=(kt == 0), stop=(kt == KT - 1),
                    )
                nc.vector.tensor_copy(
                    v_sb[:, tt, :, 0:D],
                    ps[:, 0, 0:GF].rearrange("p (h d) -> p h d", h=HPC))
            nc.gpsimd.memset(v_sb[:, 16 * b:16 * b + 16, :, D:D + 1], 1.0)

        # ---- attention -------------------------------------------------------
        # one A2A per head: the h=0 collective overlaps h=1 attention compute
        a2a_in_h = [dram.tile([NCORES, D, TSL], BF16, name=f"a2ain{h}")
                    for h in range(HPC)]
        a2a_out_h = [dram.tile([NCORES, D, TSL], BF16, name=f"a2aout{h}")
                     for h in range(HPC)]

        ones1 = pool1.tile([1, D], BF16)
        nc.gpsimd.memset(ones1[:], 1.0)
        # shifted triangular mask strip: strip[p, x] = 1.0 iff x - 384 >= p
        strip = pool1.tile([128, 896], BF16)
        nc.gpsimd.memset(strip[:], 1.0)
        nc.gpsimd.affine_select(
            out=strip[:], in_=strip[:],
            compare_op=mybir.AluOpType.is_ge, fill=0.0,
            base=-384, pattern=[[1, 896]], channel_multiplier=-1)

        # staging for all heads'/blocks' normalized attention outputs:
        # [64 part, h, slot(=4b+tb), 512] -> one DMA per h to a2a_in
        atall = pool1.tile([D, HPC, NCORES, TSL], BF16)

        pending = []

        def attn_batch(h, b):
            qrow = 64 * h
            if True:
                tok0 = b * S               # batch token offset (flattened)
                tt0 = tok0 // 128          # v tile offset
                for tb in (3, 2, 1, 0):    # big blocks first: short tail chain
                    ntj = 4 * (tb + 1)     # causal: tj tiles 0..ntj-1
                    av = psum_av.tile([D + 1, 512], F32, tag="av")
                    for tjq in range(ntj // 4):   # quads of tj tiles
                        if tjq < ntj // 4 - 1:
                            # full (strictly-lower) quad
                            st_ps = psum.tile([128, 2, 512], F32, tag="mm")
                            pt = ppool.tile([128, 4, 512], BF16, tag="pt")
                            for u in range(4):
                                t = 4 * tjq + u
                                nc.tensor.matmul(
                                    st_ps[:, u % 2, :],
                                    lhsT=qk_sb[qrow:qrow + 64, 1,
                                               tok0 + t * 128:tok0 + (t + 1) * 128],
                                    rhs=qk_sb[qrow:qrow + 64, 0,
                                              tok0 + tb * 512:tok0 + (tb + 1) * 512],
                                    start=True, stop=True,
                                )
                                # exp((q.k)/sqrt(d)); pairs share one psum tile
                                if u % 2 == 1:
                                    nc.scalar.activation(
                                        pt[:, u - 1:u + 1, :], st_ps[:],
                                        mybir.ActivationFunctionType.Exp, scale=0.125)
                                    if u == 1:
                                        st_ps = psum.tile([128, 2, 512], F32, tag="mm")
                            for u in range(4):
                                t = 4 * tjq + u
                                nc.tensor.matmul(
                                    av[:],
                                    lhsT=v_sb[:, tt0 + t, h, :],
                                    rhs=pt[:, u, :],
                                    start=(t == 0), stop=(t == ntj - 1),
                                )
                        else:
                            # diagonal quad: tile u only needs queries
                            # ti >= 128u -> widths 512/384/256/128, packed in
                            # pairs; masked via strip multiply (keep c' >= p)
                            for pair in range(2):
                                st_d = psum.tile([128, 1024], F32, tag="mm")
                                pt_d = ppool.tile([128, 1024], BF16, tag="pt")
                                w0 = 512 - 128 * (2 * pair)
                                w1 = 512 - 128 * (2 * pair + 1)
                                for u2 in range(2):
                                    u = 2 * pair + u2
                                    t = 4 * tjq + u
                                    w = 512 - 128 * u
                                    off = 0 if u2 == 0 else w0
                                    qlo = tok0 + tb * 512 + 128 * u
                                    nc.tensor.matmul(
                                        st_d[:, off:off + w],
                                        lhsT=qk_sb[qrow:qrow + 64, 1,
                                                   tok0 + t * 128:tok0 + (t + 1) * 128],
                                        rhs=qk_sb[qrow:qrow + 64, 0, qlo:qlo + w],
                                        start=True, stop=True,
                                    )
                                nc.scalar.activation(
                                    pt_d[:, 0:w0 + w1], st_d[:, 0:w0 + w1],
                                    mybir.ActivationFunctionType.Exp, scale=0.125)
                                for u2 in range(2):
                                    u = 2 * pair + u2
                                    t = 4 * tjq + u
                                    w = 512 - 128 * u
                                    off = 0 if u2 == 0 else w0
                                    nc.vector.tensor_mul(
                                        pt_d[:, off:off + w], pt_d[:, off:off + w],
                                        strip[:, 384:384 + w])
                                    nc.tensor.matmul(
                                        av[:, 128 * u:512],
                                        lhsT=v_sb[:, tt0 + t, h, :],
                                        rhs=pt_d[:, off:off + w],
                                        start=(t == 0), stop=(t == ntj - 1),
                                    )
                        if pending:
                            pending.pop(0)()
                    # normalize by row D (the P row-sums); recip now (DVE),
                    # but defer the PE broadcast+mul until after the next
                    # block's S quads so PE never waits on DVE here
                    recip = small.tile([1, 512], BF16, tag="recip")
                    with nc.allow_low_precision("softmax recip bf16 is fine"):
                        nc.vector.reciprocal(recip[:], av[D:D + 1, :])

                    def make_epilogue(av=av, recip=recip, h=h, slot=4 * b + tb):
                        def epi():
                            rb = psum_rb.tile([D, 512], F32, tag="rb")
                            nc.tensor.matmul(rb[:], lhsT=ones1[:], rhs=recip[:],
                                             start=True, stop=True)
                            rb_sb = ppool.tile([D, 512], F32, tag="rbs")
                            nc.vector.tensor_copy(rb_sb[:], rb[:])
                            dst = atall[:, h, slot, :]
                            if zero_attn_bias:
                                nc.vector.tensor_mul(dst, av[0:D, :], rb_sb[:])
                            else:
                                at = ppool.tile([D, 512], BF16, tag="at")
                                nc.vector.tensor_mul(at[:], av[0:D, :], rb_sb[:])
                                nc.scalar.activation(
                                    dst, at[:],
                                    mybir.ActivationFunctionType.Identity,
                                    bias=bv_t[:, h, :])
                        return epi
                    pending.append(make_epilogue())

        def head_a2a(h):
            while pending:
                pending.pop(0)()
            # per-head store + AllToAll: [64, slot, 512] -> a2a_in_h[slot, :, :]
            nc.sync.dma_start(
                a2a_in_h[h][:].rearrange("s p c -> p s c"),
                atall[:, h, :, :])
            nc.gpsimd.collective_compute(
                "AllToAll",
                mybir.AluOpType.bypass,
                ins=[a2a_in_h[h].opt()],
                outs=[a2a_out_h[h].opt()],
                replica_groups=[list(range(NCORES))],
            )

        # schedule: proj(b0); attn(h0,b0) exps overlap proj(b1) on ACT/PE;
        # A2A#1 (head 0) overlaps all of head 1's attention; A2A#2 tails.
        project_batch(0)
        attn_batch(0, 0)
        project_batch(1)
        attn_batch(0, 1)
        head_a2a(0)
        attn_batch(1, 0)
        attn_batch(1, 1)
        head_a2a(1)

        # ---- c_proj over the received [NX, TSL] block -----------------------
        wproj_bf = pool1.tile([128, KT, NX], BF16)
        for half in range(2):
            sl = slice(half * (KT // 2), (half + 1) * (KT // 2))
            nc.sync.dma_start(
                wproj_bf[:, sl, :],
                wproj[:, :].rearrange("(kt p) c -> p kt c", p=128)[:, sl, :])
        # feature rows of art: partition 0:64 <- head parity 0, 64:128 <- parity 1
        art_bf = pool1.tile([128, KT, TSL], BF16)
        for h in range(HPC):
            nc.sync.dma_start(
                art_bf[h * D:(h + 1) * D, :, :],
                a2a_out_h[h][:].rearrange("s p c -> p s c"))
        otall = pool1.tile([128, KT, TSL], F32)
        for ntile in range(KT):
            ps = psum.tile([128, 2, 512], F32, tag="mm")
            for ft in range(KT):
                nc.tensor.matmul(
                    ps[:, 0, :],
                    lhsT=wproj_bf[:, ft, ntile * 128:(ntile + 1) * 128],
                    rhs=art_bf[:, ft, :],
                    start=(ft == 0), stop=(ft == KT - 1),
                )
            if zero_proj_bias:
                nc.vector.tensor_copy(otall[:, ntile, :], ps[:, 0, :])
            else:
                nc.scalar.activation(
                    otall[:, ntile, :], ps[:, 0, :],
                    mybir.ActivationFunctionType.Identity, bias=bproj_t[:, ntile, :])
            nc.sync.dma_start(
                out_ext[ntile * 128:(ntile + 1) * 128, :], otall[:, ntile, :])

    nc.finalize()
    return nc


_CACHE = {}


def _get_nc(zero_attn_bias, zero_proj_bias):
    key = (zero_attn_bias, zero_proj_bias)
    if key not in _CACHE:
        _CACHE[key] = build(*key)
    return _CACHE[key]


def kernel(hidden_states, c_attn_w, c_attn_b, c_proj_w, c_proj_b, **extra):
    hidden_states = np.asarray(hidden_states, np.float32)
    c_attn_w = np.asarray(c_attn_w, np.float32)
    c_attn_b = np.asarray(c_attn_b, np.float32)
    c_proj_w = np.asarray(c_proj_w, np.float32)
    c_proj_b = np.asarray(c_proj_b, np.float32)

    zero_attn_bias = not np.any(c_attn_b)
    zero_proj_bias = not np.any(c_proj_b)
    nc = _get_nc(zero_attn_bias, zero_proj_bias)

    bf = ml_dtypes.bfloat16
    # [NX, B*S] pre-transposed hidden states in the kernel's compute dtype
    hsT = np.ascontiguousarray(hidden_states.reshape(B * S, NX).T).astype(bf)
    wproj_bf = np.ascontiguousarray(c_proj_w).astype(bf)
    bproj = np.ascontiguousarray(c_proj_b.reshape(NX, 1))

    in_maps = []
    for i in range(NCORES):
        cols = np.r_[i * GF:(i + 1) * GF,
                     NX + i * GF:NX + (i + 1) * GF,
                     2 * NX + i * GF:2 * NX + (i + 1) * GF]
        in_maps.append({
            "hst": hsT,
            "wqkv": np.ascontiguousarray(c_attn_w[:, cols]).astype(bf),
            "bqkv": np.ascontiguousarray(c_attn_b[cols].reshape(3 * GF, 1)),
            "wproj": wproj_bf,
            "bproj": bproj,
        })

    res = run_bass_kernel_spmd(nc, in_maps, core_ids=list(range(NCORES)))
    out = np.empty((B * S, NX), np.float32)
    for i in range(NCORES):
        out[i * TSL:(i + 1) * TSL, :] = res.results[i]["out"].T
    return out.reshape(B, S, NX)


if __name__ == "__main__":
    rng = np.random.default_rng(0)
    hs = rng.standard_normal((B, S, NX), dtype=np.float32)
    wa = (rng.standard_normal((NX, 3 * NX), dtype=np.float32) * 0.02)
    wp = (rng.standard_normal((NX, NX), dtype=np.float32) * 0.02)
    o = kernel(hidden_states=hs, c_attn_w=wa, c_attn_b=np.zeros(3 * NX, np.float32),
               c_proj_w=wp, c_proj_b=np.zeros(NX, np.float32))
    print(o.shape, o.dtype)



# revision 11
# speedup vs baseline: 1.2764x; 1.2764x over previous
"""Distributed GPT-2 attention block for 8 TRN2 NeuronCores — collective-free.

Sharding: core c handles heads {2c, 2c+1} for BOTH batches (tensor-parallel
column split of c_attn). Each core then computes a PARTIAL c_proj over all
4096 tokens using only its 128 feature rows of c_proj; the host sums the 8
partial outputs (the tensor-parallel all-reduce done at unshard time) and
adds the bias. No on-device collectives -> no cross-core barrier, no
launch-skew sensitivity.

Layout: hidden_states pre-transposed [NX, B*S] bf16; q,k kept in [d, t]
layout so scores come out as [keys, queries]; v natural [t, d] with an
appended ones-column so the PV matmul also yields softmax denominators.
Score matmuls for the two heads are row-packed (head 0 in PE rows 0-63,
head 1 in rows 64-127) into one 2-bank PSUM tile, so they run concurrently
and a single ACT pass exps both heads. Causal masking = block skipping +
width-shrunk diagonal tiles + a [128,128] triangular mask multiply.
Softmax normalization: reciprocal_approx_fast on the denominator rows, one
K=2 matmul broadcast for both heads, two DVE muls. All matmuls accumulate
in f32 PSUM; compute dtype bf16.
"""

import numpy as np
import ml_dtypes
from contextlib import ExitStack

import concourse.bass as bass
import concourse.bacc as bacc
import concourse.mybir as mybir
import concourse.tile as tile
from concourse.bass_utils import run_bass_kernel_spmd

B, S, NX = 2, 2048, 1024
H, D = 16, 64
HPC = 2              # heads per core
GF = HPC * D         # 128 features per head group
NCORES = 8
SF = B * S           # 4096 flattened tokens
KT = NX // 128       # 8 contraction tiles of the 1024 feature axis

F32 = mybir.dt.float32
BF16 = mybir.dt.bfloat16


def build(zero_attn_bias: bool) -> bass.Bass:
    nc = bacc.Bacc(None)

    hst = nc.declare_dram_parameter("hst", [NX, SF], BF16, isOutput=False)
    wqkv = nc.declare_dram_parameter("wqkv", [NX, 3 * GF], BF16, isOutput=False)
    bqkv = nc.declare_dram_parameter("bqkv", [3 * GF, 1], F32, isOutput=False)
    wproj = nc.declare_dram_parameter("wproj", [GF, NX], BF16, isOutput=False)
    out_ext = nc.declare_dram_parameter("out", [NX, SF], BF16, isOutput=True)

    with tile.TileContext(nc) as tc, ExitStack() as ctx:
        pool1 = ctx.enter_context(tc.tile_pool(name="persist", bufs=1))
        small = ctx.enter_context(tc.tile_pool(name="small", bufs=2))
        ppool = ctx.enter_context(tc.tile_pool(name="ppool", bufs=3))
        opool = ctx.enter_context(tc.tile_pool(name="opool", bufs=3))
        # PSUM budget (8 banks): S pair-tiles 2x2 + av pair-tile 1x2 + pq 2x1
        psum_s = ctx.enter_context(tc.tile_pool(name="psum_s", bufs=2, space="PSUM"))
        psum_av = ctx.enter_context(tc.tile_pool(name="psum_av", bufs=1, space="PSUM"))
        psum_pq = ctx.enter_context(tc.tile_pool(name="psum_pq", bufs=2, space="PSUM"))

        # ---- load weights and hidden states ---------------------------------
        wqkv_bf = pool1.tile([128, KT, 3 * GF], BF16)
        nc.sync.dma_start(
            wqkv_bf[:], wqkv[:, :].rearrange("(kt p) c -> p kt c", p=128))
        wproj_bf = pool1.tile([GF, NX], BF16)
        nc.sync.dma_start(wproj_bf[:], wproj[:, :])
        hst_bf = pool1.tile([128, KT, SF], BF16)
        for q in range(4):
            tsl = slice(q * (SF // 4), (q + 1) * (SF // 4))
            for kt in range(KT):
                nc.sync.dma_start(
                    hst_bf[:, kt, tsl], hst[kt * 128:(kt + 1) * 128, tsl])

        # biases (q/k: feature-per-partition in the [d, t] layout)
        bqk_t = pool1.tile([128, 2, 1], F32)   # ft 0 = q(2 heads), ft 1 = k
        bv_t = pool1.tile([64, HPC, 1], F32)
        if not zero_attn_bias:
            for ft in range(2):
                nc.sync.dma_start(bqk_t[:, ft, :], bqkv[ft * 128:(ft + 1) * 128, :])
            for h in range(HPC):
                nc.sync.dma_start(
                    bv_t[:, h, :], bqkv[2 * GF + h * D:2 * GF + (h + 1) * D, :])

        # constant masks / helpers
        tri = pool1.tile([128, 128], BF16)     # tri[p, y] = 1.0 iff y >= p
        nc.gpsimd.memset(tri[:], 1.0)
        nc.gpsimd.affine_select(
            out=tri[:], in_=tri[:],
            compare_op=mybir.AluOpType.is_ge, fill=0.0,
            base=0, pattern=[[1, 128]], channel_multiplier=-1)
        ones1 = pool1.tile([1, D], BF16)
        nc.gpsimd.memset(ones1[:], 1.0)

        # ---- QKV projection --------------------------------------------------
        # q,k transposed: qk_sb[:, ft, t]; ft 0 = q (2 heads), ft 1 = k
        qk_sb = pool1.tile([128, 2, SF], BF16)
        # v natural [token, feat] + ones column: [128, tt, h, 65]
        v_sb = pool1.tile([128, SF // 128, HPC, D + 1], BF16)
        nc.gpsimd.memset(v_sb[:, :, :, D:D + 1], 1.0)

        def project_qk(b):
            # kt-outer, tch-pairs: one LDWEIGHTS per (ft, pair, kt) serves 2 MMs
            for ft in range(2):
                for pr in range(2):
                    tchs = [4 * b + 2 * pr, 4 * b + 2 * pr + 1]
                    ps = [psum_pq.tile([128, 512], F32, tag="pq",
                                       name=f"pqk{ft}{pr}{i}")
                          for i in range(len(tchs))]
                    for kt in range(KT):
                        for i, tch in enumerate(tchs):
                            nc.tensor.matmul(
                                ps[i][:],
                                lhsT=wqkv_bf[:, kt, ft * 128:(ft + 1) * 128],
                                rhs=hst_bf[:, kt, tch * 512:(tch + 1) * 512],
                                start=(kt == 0), stop=(kt == KT - 1),
                            )
                    for i, tch in enumerate(tchs):
                        dst = qk_sb[:, ft, tch * 512:(tch + 1) * 512]
                        if zero_attn_bias:
                            nc.vector.tensor_copy(dst, ps[i][:])
                        else:
                            nc.scalar.activation(
                                dst, ps[i][:],
                                mybir.ActivationFunctionType.Identity,
                                bias=bqk_t[:, ft, :])

        def project_v(b):
            # one accumulation group per PSUM tile: a second group sharing
            # the bank would wipe has_written on its start and drop kt=0
            for tt in range(16 * b, 16 * b + 16):
                ps = psum_pq.tile([128, GF], F32, tag="pq")
                for kt in range(KT):
                    nc.tensor.matmul(
                        ps[:],
                        lhsT=hst_bf[:, kt, tt * 128:(tt + 1) * 128],
                        rhs=wqkv_bf[:, kt, 2 * GF:3 * GF],
                        start=(kt == 0), stop=(kt == KT - 1),
                    )
                nc.vector.tensor_copy(
                    v_sb[:, tt, :, 0:D],
                    ps[:].rearrange("p (h d) -> p h d", h=HPC))

        # ---- attention -------------------------------------------------------
        # normalized attention outputs, head on the FREE axis (all DVE ops at
        # partition base 0); consolidated per-slot into atall by SBUF DMA
        at64 = pool1.tile([D, HPC, NCORES, 512], BF16)
        # c_proj rhs layout: [feat(2 heads stacked on partitions), slot, 512]
        atall = pool1.tile([128, NCORES, 512], BF16)

        pending = []

        def attn_block(b, tb):
            """One 512-query block (both heads, row-packed)."""
            tok0 = b * S
            tt0 = tok0 // 128
            q0 = tok0 + tb * 512
            av = psum_av.tile([D + 1, HPC, 512], F32, tag="av")
            ntj = 4 * (tb + 1)
            for t in range(ntj):
                u = t - 4 * tb              # >=0 only inside diagonal quad
                w = 512 if u < 0 else 512 - 128 * u
                c0 = 512 - w                # query-col offset of this tile
                st = psum_s.tile([128, HPC, 512], F32, tag="st")
                for h in range(HPC):        # row-packed pair: concurrent MMs
                    nc.tensor.matmul(
                        st[:, h, c0:512],
                        lhsT=qk_sb[64 * h:64 * h + 64, 1,
                                   tok0 + t * 128:tok0 + (t + 1) * 128],
                        rhs=qk_sb[64 * h:64 * h + 64, 0, q0 + c0:q0 + 512],
                        start=True, stop=True,
                    )
                if pending and t == 0:
                    # prev block's epilogue: its rb matmul must precede this
                    # block's first PV on the PE queue (av slot WAR)
                    pending.pop(0)()
                pt = ppool.tile([128, HPC, 512], BF16, tag="pt")
                nc.scalar.activation(
                    pt[:, :, c0:512], st[:, :, c0:512],
                    mybir.ActivationFunctionType.Exp, scale=0.125)
                if u >= 0:
                    # triangular boundary on the first 128 query cols
                    for h in range(HPC):
                        nc.vector.tensor_mul(
                            pt[:, h, c0:c0 + 128], pt[:, h, c0:c0 + 128],
                            tri[:])
                for h in range(HPC):
                    nc.tensor.matmul(
                        av[:, h, c0:512],
                        lhsT=v_sb[:, tt0 + t, h, :],
                        rhs=pt[:, h, c0:512],
                        start=(t == 0), stop=(t == ntj - 1),
                    )

            # snapshot numerators + denominators out of PSUM at block end so
            # the av slot frees immediately (next block's PV can't race the
            # deferred epilogue)
            avs = ppool.tile([D, HPC, 512], BF16, tag="avs")
            nc.vector.tensor_copy(avs[:], av[0:D, :, :])
            r2 = small.tile([1, HPC, 512], BF16, tag="r2")
            with nc.allow_low_precision("softmax recip bf16 is fine"):
                nc.vector.reciprocal(r2[:], av[D:D + 1, :, :])

            def make_epilogue(avs=avs, r2=r2, slot=4 * b + tb):
                def epi():
                    for h in range(HPC):
                        rb = psum_pq.tile([D, 512], F32, tag="pq",
                                          name=f"rb{h}")
                        nc.tensor.matmul(rb[:], lhsT=ones1[:],
                                         rhs=r2[:, h, :],
                                         start=True, stop=True)
                        dst = at64[:, h, slot, :]
                        if zero_attn_bias:
                            nc.vector.tensor_mul(dst, avs[:, h, :], rb[:])
                        else:
                            at = ppool.tile([D, 512], BF16, tag="at")
                            nc.vector.tensor_mul(at[:], avs[:, h, :], rb[:])
                            nc.scalar.activation(
                                dst, at[:],
                                mybir.ActivationFunctionType.Identity,
                                bias=bv_t[:, h, :])
                        nc.sync.dma_start(
                            atall[64 * h:64 * h + 64, slot, :], dst)
                return epi
            pending.append(make_epilogue())

        def flush_pending():
            while pending:
                pending.pop(0)()

        def proj_slots(slots, nrng):
            # partial c_proj: out[128n : , 512s : ] = wproj[:, ncols].T @ atall
            for n in nrng:
                for s in slots:
                    ps = psum_pq.tile([128, 512], F32, tag="pq")
                    nc.tensor.matmul(
                        ps[:],
                        lhsT=wproj_bf[:, n * 128:(n + 1) * 128],
                        rhs=atall[:, s, :],
                        start=True, stop=True,
                    )
                    ot = opool.tile([128, 512], BF16, tag="ot")
                    nc.vector.tensor_copy(ot[:], ps[:])
                    nc.sync.dma_start(
                        out_ext[n * 128:(n + 1) * 128, s * 512:(s + 1) * 512],
                        ot[:])

        # ---- schedule --------------------------------------------------------
        project_qk(0)
        project_v(0)
        attn_block(0, 3)
        attn_block(0, 2)
        project_qk(1)           # fills PE gaps while attn(b0) is ACT-bound
        attn_block(0, 1)
        project_v(1)
        attn_block(0, 0)
        attn_block(1, 3)
        proj_slots([3, 2], range(8))   # b0 slots stream out during attn(b1)
        attn_block(1, 2)
        proj_slots([1, 0], range(8))
        attn_block(1, 1)
        attn_block(1, 0)
        flush_pending()
        proj_slots([7, 6, 5, 4], range(8))

    nc.finalize()
    return nc


_CACHE = {}


def _get_nc(zero_attn_bias):
    if zero_attn_bias not in _CACHE:
        _CACHE[zero_attn_bias] = build(zero_attn_bias)
    return _CACHE[zero_attn_bias]


def kernel(hidden_states, c_attn_w, c_attn_b, c_proj_w, c_proj_b, **extra):
    hidden_states = np.asarray(hidden_states, np.float32)
    c_attn_w = np.asarray(c_attn_w, np.float32)
    c_attn_b = np.asarray(c_attn_b, np.float32)
    c_proj_w = np.asarray(c_proj_w, np.float32)
    c_proj_b = np.asarray(c_proj_b, np.float32)

    zero_attn_bias = not np.any(c_attn_b)
    nc = _get_nc(zero_attn_bias)

    bf = ml_dtypes.bfloat16
    hsT = np.ascontiguousarray(hidden_states.reshape(B * S, NX).T).astype(bf)

    in_maps = []
    for i in range(NCORES):
        cols = np.r_[i * GF:(i + 1) * GF,
                     NX + i * GF:NX + (i + 1) * GF,
                     2 * NX + i * GF:2 * NX + (i + 1) * GF]
        in_maps.append({
            "hst": hsT,
            "wqkv": np.ascontiguousarray(c_attn_w[:, cols]).astype(bf),
            "bqkv": np.ascontiguousarray(c_attn_b[cols].reshape(3 * GF, 1)),
            "wproj": np.ascontiguousarray(
                c_proj_w[i * GF:(i + 1) * GF, :]).astype(bf),
        })

    res = run_bass_kernel_spmd(nc, in_maps, core_ids=list(range(NCORES)))
    acc = np.zeros((NX, B * S), np.float32)
    for i in range(NCORES):
        acc += np.asarray(res.results[i]["out"]).astype(np.float32)
    out = acc.T + c_proj_b[None, :]
    return np.ascontiguousarray(out.reshape(B, S, NX))


if __name__ == "__main__":
    rng = np.random.default_rng(0)
    hs = rng.standard_normal((B, S, NX), dtype=np.float32)
    wa = (rng.standard_normal((NX, 3 * NX), dtype=np.float32) * 0.02)
    wp = (rng.standard_normal((NX, NX), dtype=np.float32) * 0.02)
    o = kernel(hidden_states=hs, c_attn_w=wa, c_attn_b=np.zeros(3 * NX, np.float32),
               c_proj_w=wp, c_proj_b=np.zeros(NX, np.float32))
    print(o.shape, o.dtype)


# revision 17
# speedup vs baseline: 1.2859x; 1.0075x over previous
"""Distributed GPT-2 attention block for 8 TRN2 NeuronCores — collective-free.

Sharding: core c handles heads {2c, 2c+1} for BOTH batches (tensor-parallel
column split of c_attn). Each core then computes a PARTIAL c_proj over all
4096 tokens using only its 128 feature rows of c_proj; the host sums the 8
partial outputs (the tensor-parallel all-reduce done at unshard time) and
adds the bias. No on-device collectives -> no cross-core barrier, no
launch-skew sensitivity.

Layout: hidden_states pre-transposed [NX, B*S] bf16; q,k kept in [d, t]
layout so scores come out as [keys, queries]; v natural [t, d] with an
appended ones-column so the PV matmul also yields softmax denominators.
Score matmuls for the two heads are row-packed (head 0 in PE rows 0-63,
head 1 in rows 64-127) into one 2-bank PSUM tile, so they run concurrently
and a single ACT pass exps both heads. Causal masking = block skipping +
width-shrunk diagonal tiles + a [128,128] triangular mask multiply.
Softmax normalization: reciprocal_approx_fast on the denominator rows, one
K=2 matmul broadcast for both heads, two DVE muls. All matmuls accumulate
in f32 PSUM; compute dtype bf16.
"""

import numpy as np
import ml_dtypes
from contextlib import ExitStack

import concourse.bass as bass
import concourse.bacc as bacc
import concourse.mybir as mybir
import concourse.tile as tile
from concourse.bass_utils import run_bass_kernel_spmd

B, S, NX = 2, 2048, 1024
H, D = 16, 64
HPC = 2              # heads per core
GF = HPC * D         # 128 features per head group
NCORES = 8
SF = B * S           # 4096 flattened tokens
KT = NX // 128       # 8 contraction tiles of the 1024 feature axis

F32 = mybir.dt.float32
BF16 = mybir.dt.bfloat16


def build(zero_attn_bias: bool) -> bass.Bass:
    nc = bacc.Bacc(None)

    hst = nc.declare_dram_parameter("hst", [NX, SF], BF16, isOutput=False)
    wqkv = nc.declare_dram_parameter("wqkv", [NX, 3 * GF], BF16, isOutput=False)
    bqkv = nc.declare_dram_parameter("bqkv", [3 * GF, 1], F32, isOutput=False)
    wproj = nc.declare_dram_parameter("wproj", [GF, NX], BF16, isOutput=False)
    out_ext = nc.declare_dram_parameter("out", [NX, SF], BF16, isOutput=True)

    with tile.TileContext(nc) as tc, ExitStack() as ctx:
        pool1 = ctx.enter_context(tc.tile_pool(name="persist", bufs=1))
        small = ctx.enter_context(tc.tile_pool(name="small", bufs=2))
        ppool = ctx.enter_context(tc.tile_pool(name="ppool", bufs=3))
        opool = ctx.enter_context(tc.tile_pool(name="opool", bufs=3))
        # PSUM budget (8 banks): S pair-tiles 2x2 + av pair-tile 1x2 + pq 2x1
        psum_s = ctx.enter_context(tc.tile_pool(name="psum_s", bufs=2, space="PSUM"))
        psum_av = ctx.enter_context(tc.tile_pool(name="psum_av", bufs=1, space="PSUM"))
        psum_pq = ctx.enter_context(tc.tile_pool(name="psum_pq", bufs=2, space="PSUM"))

        # ---- load weights and hidden states ---------------------------------
        wqkv_bf = pool1.tile([128, KT, 3 * GF], BF16)
        nc.sync.dma_start(
            wqkv_bf[:], wqkv[:, :].rearrange("(kt p) c -> p kt c", p=128))
        wproj_bf = pool1.tile([GF, NX], BF16)
        nc.sync.dma_start(wproj_bf[:], wproj[:, :])
        hst_bf = pool1.tile([128, KT, SF], BF16)
        for q in range(4):
            tsl = slice(q * (SF // 4), (q + 1) * (SF // 4))
            for kt in range(KT):
                nc.sync.dma_start(
                    hst_bf[:, kt, tsl], hst[kt * 128:(kt + 1) * 128, tsl])

        # biases (q/k: feature-per-partition in the [d, t] layout)
        bqk_t = pool1.tile([128, 2, 1], F32)   # ft 0 = q(2 heads), ft 1 = k
        bv_t = pool1.tile([64, HPC, 1], F32)
        if not zero_attn_bias:
            for ft in range(2):
                nc.sync.dma_start(bqk_t[:, ft, :], bqkv[ft * 128:(ft + 1) * 128, :])
            for h in range(HPC):
                nc.sync.dma_start(
                    bv_t[:, h, :], bqkv[2 * GF + h * D:2 * GF + (h + 1) * D, :])

        # constant masks / helpers
        tri = pool1.tile([128, 128], BF16)     # tri[p, y] = 1.0 iff y >= p
        nc.gpsimd.memset(tri[:], 1.0)
        nc.gpsimd.affine_select(
            out=tri[:], in_=tri[:],
            compare_op=mybir.AluOpType.is_ge, fill=0.0,
            base=0, pattern=[[1, 128]], channel_multiplier=-1)
        ones1 = pool1.tile([1, D], BF16)
        nc.gpsimd.memset(ones1[:], 1.0)

        # ---- QKV projection --------------------------------------------------
        # q,k transposed: qk_sb[:, ft, t]; ft 0 = q (2 heads), ft 1 = k
        qk_sb = pool1.tile([128, 2, SF], BF16)
        # v natural [token, feat] + ones column: [128, tt, h, 65]
        v_sb = pool1.tile([128, SF // 128, HPC, D + 1], BF16)
        nc.gpsimd.memset(v_sb[:, :, :, D:D + 1], 1.0)

        def project_qk(b):
            # kt-outer, tch-pairs: one LDWEIGHTS per (ft, pair, kt) serves 2 MMs
            for ft in range(2):
                for pr in range(2):
                    tchs = [4 * b + 2 * pr, 4 * b + 2 * pr + 1]
                    ps = [psum_pq.tile([128, 512], F32, tag="pq",
                                       name=f"pqk{ft}{pr}{i}")
                          for i in range(len(tchs))]
                    for kt in range(KT):
                        for i, tch in enumerate(tchs):
                            nc.tensor.matmul(
                                ps[i][:],
                                lhsT=wqkv_bf[:, kt, ft * 128:(ft + 1) * 128],
                                rhs=hst_bf[:, kt, tch * 512:(tch + 1) * 512],
                                start=(kt == 0), stop=(kt == KT - 1),
                            )
                    for i, tch in enumerate(tchs):
                        dst = qk_sb[:, ft, tch * 512:(tch + 1) * 512]
                        if zero_attn_bias:
                            nc.vector.tensor_copy(dst, ps[i][:])
                        else:
                            nc.scalar.activation(
                                dst, ps[i][:],
                                mybir.ActivationFunctionType.Identity,
                                bias=bqk_t[:, ft, :])

        def project_v(b):
            # one accumulation group per PSUM tile: a second group sharing
            # the bank would wipe has_written on its start and drop kt=0
            for tt in range(16 * b, 16 * b + 16):
                ps = psum_pq.tile([128, GF], F32, tag="pq")
                for kt in range(KT):
                    nc.tensor.matmul(
                        ps[:],
                        lhsT=hst_bf[:, kt, tt * 128:(tt + 1) * 128],
                        rhs=wqkv_bf[:, kt, 2 * GF:3 * GF],
                        start=(kt == 0), stop=(kt == KT - 1),
                    )
                nc.vector.tensor_copy(
                    v_sb[:, tt, :, 0:D],
                    ps[:].rearrange("p (h d) -> p h d", h=HPC))

        # ---- attention -------------------------------------------------------
        # normalized attention outputs, head on the FREE axis (all DVE ops at
        # partition base 0); consolidated per-slot into atall by SBUF DMA
        at64 = pool1.tile([D, HPC, NCORES, 512], BF16)
        # c_proj rhs layout: [feat(2 heads stacked on partitions), slot, 512]
        atall = pool1.tile([128, NCORES, 512], BF16)

        pending = []

        def attn_block(b, tb):
            """One 512-query block (both heads, row-packed)."""
            tok0 = b * S
            tt0 = tok0 // 128
            q0 = tok0 + tb * 512
            av = psum_av.tile([D + 1, HPC, 512], F32, tag="av")
            ntj = 4 * (tb + 1)
            for t in range(ntj):
                u = t - 4 * tb              # >=0 only inside diagonal quad
                w = 512 if u < 0 else 512 - 128 * u
                c0 = 512 - w                # query-col offset of this tile
                st = psum_s.tile([128, HPC, 512], F32, tag="st")
                for h in range(HPC):        # row-packed pair: concurrent MMs
                    nc.tensor.matmul(
                        st[:, h, c0:512],
                        lhsT=qk_sb[64 * h:64 * h + 64, 1,
                                   tok0 + t * 128:tok0 + (t + 1) * 128],
                        rhs=qk_sb[64 * h:64 * h + 64, 0, q0 + c0:q0 + 512],
                        start=True, stop=True,
                    )
                if pending and t == 3:
                    pending.pop(0)()
                pt = ppool.tile([128, HPC, 512], BF16, tag="pt")
                nc.scalar.activation(
                    pt[:, :, c0:512], st[:, :, c0:512],
                    mybir.ActivationFunctionType.Exp, scale=0.125)
                if u >= 0:
                    # triangular boundary on the first 128 query cols
                    for h in range(HPC):
                        nc.vector.tensor_mul(
                            pt[:, h, c0:c0 + 128], pt[:, h, c0:c0 + 128],
                            tri[:])
                for h in range(HPC):
                    nc.tensor.matmul(
                        av[:, h, c0:512],
                        lhsT=v_sb[:, tt0 + t, h, :],
                        rhs=pt[:, h, c0:512],
                        start=(t == 0), stop=(t == ntj - 1),
                    )

            # snapshot numerators + denominators out of PSUM at block end so
            # the av slot frees immediately (next block's PV can't race the
            # deferred epilogue)
            avs = ppool.tile([D, HPC, 512], BF16, tag="avs")
            nc.vector.tensor_copy(avs[:], av[0:D, :, :])
            r2 = small.tile([1, HPC, 512], BF16, tag="r2")
            with nc.allow_low_precision("softmax recip bf16 is fine"):
                for h in range(HPC):
                    nc.vector.reciprocal(r2[:, h, :], av[D:D + 1, h, :])

            def make_epilogue(avs=avs, r2=r2, slot=4 * b + tb):
                def epi():
                    for h in range(HPC):
                        rb = psum_pq.tile([D, 512], F32, tag="pq",
                                          name=f"rb{h}")
                        nc.tensor.matmul(rb[:], lhsT=ones1[:],
                                         rhs=r2[:, h, :],
                                         start=True, stop=True)
                        rbs = ppool.tile([D, 512], BF16, tag="rbs",
                                         name=f"rbs{h}")
                        nc.vector.tensor_copy(rbs[:], rb[:])
                        dst = at64[:, h, slot, :]
                        if zero_attn_bias:
                            nc.vector.tensor_mul(dst, avs[:, h, :], rbs[:])
                        else:
                            at = ppool.tile([D, 512], BF16, tag="at")
                            nc.vector.tensor_mul(at[:], avs[:, h, :], rbs[:])
                            nc.scalar.activation(
                                dst, at[:],
                                mybir.ActivationFunctionType.Identity,
                                bias=bv_t[:, h, :])
                        nc.sync.dma_start(
                            atall[64 * h:64 * h + 64, slot, :], dst)
                return epi
            pending.append(make_epilogue())

        def proj_slots(slots, nrng, mix=False):
            # partial c_proj: out[128n : , 512s : ] = wproj[:, ncols].T @ atall
            for n in nrng:
                for s in slots:
                    ps = psum_pq.tile([128, 512], F32, tag="pq")
                    nc.tensor.matmul(
                        ps[:],
                        lhsT=wproj_bf[:, n * 128:(n + 1) * 128],
                        rhs=atall[:, s, :],
                        start=True, stop=True,
                    )
                    ot = opool.tile([128, 512], BF16, tag="ot")
                    if mix and n % 2 == 1:
                        # tail phase: ACT is idle, split the cast load
                        nc.scalar.activation(
                            ot[:], ps[:],
                            mybir.ActivationFunctionType.Identity)
                    else:
                        nc.vector.tensor_copy(ot[:], ps[:])
                    nc.sync.dma_start(
                        out_ext[n * 128:(n + 1) * 128, s * 512:(s + 1) * 512],
                        ot[:])

        # ---- schedule --------------------------------------------------------
        project_qk(0)
        project_v(0)
        attn_block(0, 3)
        attn_block(0, 2)
        project_qk(1)           # fills PE gaps while attn(b0) is ACT-bound
        attn_block(0, 1)
        project_v(1)
        attn_block(0, 0)
        attn_block(1, 3)
        proj_slots([3, 2], range(8))   # b0 slots stream out during attn(b1)
        attn_block(1, 2)
        proj_slots([1, 0], range(8))
        attn_block(1, 1)
        proj_slots([7], range(8))
        attn_block(1, 0)
        proj_slots([6], range(8))
        while pending:
            pending.pop(0)()
        proj_slots([5, 4], range(8), mix=True)

    nc.finalize()
    return nc


_CACHE = {}


def _get_nc(zero_attn_bias):
    if zero_attn_bias not in _CACHE:
        _CACHE[zero_attn_bias] = build(zero_attn_bias)
    return _CACHE[zero_attn_bias]


def kernel(hidden_states, c_attn_w, c_attn_b, c_proj_w, c_proj_b, **extra):
    hidden_states = np.asarray(hidden_states, np.float32)
    c_attn_w = np.asarray(c_attn_w, np.float32)
    c_attn_b = np.asarray(c_attn_b, np.float32)
    c_proj_w = np.asarray(c_proj_w, np.float32)
    c_proj_b = np.asarray(c_proj_b, np.float32)

    zero_attn_bias = not np.any(c_attn_b)
    nc = _get_nc(zero_attn_bias)

    bf = ml_dtypes.bfloat16
    hsT = np.ascontiguousarray(hidden_states.reshape(B * S, NX).T).astype(bf)

    in_maps = []
    for i in range(NCORES):
        cols = np.r_[i * GF:(i + 1) * GF,
                     NX + i * GF:NX + (i + 1) * GF,
                     2 * NX + i * GF:2 * NX + (i + 1) * GF]
        in_maps.append({
            "hst": hsT,
            "wqkv": np.ascontiguousarray(c_attn_w[:, cols]).astype(bf),
            "bqkv": np.ascontiguousarray(c_attn_b[cols].reshape(3 * GF, 1)),
            "wproj": np.ascontiguousarray(
                c_proj_w[i * GF:(i + 1) * GF, :]).astype(bf),
        })

    res = run_bass_kernel_spmd(nc, in_maps, core_ids=list(range(NCORES)))
    acc = np.zeros((NX, B * S), np.float32)
    for i in range(NCORES):
        acc += np.asarray(res.results[i]["out"]).astype(np.float32)
    out = acc.T + c_proj_b[None, :]
    return np.ascontiguousarray(out.reshape(B, S, NX))


if __name__ == "__main__":
    rng = np.random.default_rng(0)
    hs = rng.standard_normal((B, S, NX), dtype=np.float32)
    wa = (rng.standard_normal((NX, 3 * NX), dtype=np.float32) * 0.02)
    wp = (rng.standard_normal((NX, NX), dtype=np.float32) * 0.02)
    o = kernel(hidden_states=hs, c_attn_w=wa, c_attn_b=np.zeros(3 * NX, np.float32),
               c_proj_w=wp, c_proj_b=np.zeros(NX, np.float32))
    print(o.shape, o.dtype)


# revision 18
# speedup vs baseline: 1.6376x; 1.2735x over previous
"""Distributed GPT-2 attention block for 8 TRN2 NeuronCores — collective-free.

Sharding: core c handles heads {2c, 2c+1} for BOTH batches (tensor-parallel
column split of c_attn). Each core then computes a PARTIAL c_proj over all
4096 tokens using only its 128 feature rows of c_proj; the host sums the 8
partial outputs (the tensor-parallel all-reduce done at unshard time) and
adds the bias. No on-device collectives -> no cross-core barrier, no
launch-skew sensitivity.

Layout: hidden_states pre-transposed [NX, B*S] bf16; q,k kept in [d, t]
layout so scores come out as [keys, queries]; v natural [t, d] with an
appended ones-column so the PV matmul also yields softmax denominators.
Score matmuls for the two heads are row-packed (head 0 in PE rows 0-63,
head 1 in rows 64-127) into one 2-bank PSUM tile, so they run concurrently
and a single ACT pass exps both heads. Causal masking = block skipping +
width-shrunk diagonal tiles + a [128,128] triangular mask multiply.
Softmax normalization: reciprocal_approx_fast on the denominator rows, one
K=2 matmul broadcast for both heads, two DVE muls. All matmuls accumulate
in f32 PSUM; compute dtype bf16.
"""

import numpy as np
import ml_dtypes
from contextlib import ExitStack

import concourse.bass as bass
import concourse.bacc as bacc
import concourse.mybir as mybir
import concourse.tile as tile
from concourse.bass_utils import run_bass_kernel_spmd

B, S, NX = 2, 2048, 1024
H, D = 16, 64
HPC = 2              # heads per core
GF = HPC * D         # 128 features per head group
NCORES = 8
SF = B * S           # 4096 flattened tokens
KT = NX // 128       # 8 contraction tiles of the 1024 feature axis

F32 = mybir.dt.float32
BF16 = mybir.dt.bfloat16


def build(zero_attn_bias: bool) -> bass.Bass:
    nc = bacc.Bacc(None)

    hst = nc.declare_dram_parameter("hst", [NX, SF], BF16, isOutput=False)
    wqkv = nc.declare_dram_parameter("wqkv", [NX, 3 * GF], BF16, isOutput=False)
    bqkv = nc.declare_dram_parameter("bqkv", [3 * GF, 1], F32, isOutput=False)
    wproj = nc.declare_dram_parameter("wproj", [GF, NX], BF16, isOutput=False)
    out_ext = nc.declare_dram_parameter("out", [NX, SF], BF16, isOutput=True)

    with tile.TileContext(nc) as tc, ExitStack() as ctx:
        pool1 = ctx.enter_context(tc.tile_pool(name="persist", bufs=1))
        small = ctx.enter_context(tc.tile_pool(name="small", bufs=2))
        ppool = ctx.enter_context(tc.tile_pool(name="ppool", bufs=3))
        opool = ctx.enter_context(tc.tile_pool(name="opool", bufs=3))
        # PSUM budget (8 banks): S pair-tiles 2x2 + av pair-tile 1x2 + pq 2x1
        psum_s = ctx.enter_context(tc.tile_pool(name="psum_s", bufs=2, space="PSUM"))
        psum_av = ctx.enter_context(tc.tile_pool(name="psum_av", bufs=1, space="PSUM"))
        psum_pq = ctx.enter_context(tc.tile_pool(name="psum_pq", bufs=2, space="PSUM"))

        # ---- load weights and hidden states ---------------------------------
        wqkv_bf = pool1.tile([128, KT, 3 * GF], BF16)
        nc.sync.dma_start(
            wqkv_bf[:], wqkv[:, :].rearrange("(kt p) c -> p kt c", p=128))
        wproj_bf = pool1.tile([GF, NX], BF16)
        nc.sync.dma_start(wproj_bf[:], wproj[:, :])
        hst_bf = pool1.tile([128, KT, SF], BF16)
        for q in range(4):
            tsl = slice(q * (SF // 4), (q + 1) * (SF // 4))
            for kt in range(KT):
                nc.sync.dma_start(
                    hst_bf[:, kt, tsl], hst[kt * 128:(kt + 1) * 128, tsl])

        # biases (q/k: feature-per-partition in the [d, t] layout)
        bqk_t = pool1.tile([128, 2, 1], F32)   # ft 0 = q(2 heads), ft 1 = k
        bv_t = pool1.tile([64, HPC, 1], F32)
        if not zero_attn_bias:
            for ft in range(2):
                nc.sync.dma_start(bqk_t[:, ft, :], bqkv[ft * 128:(ft + 1) * 128, :])
            for h in range(HPC):
                nc.sync.dma_start(
                    bv_t[:, h, :], bqkv[2 * GF + h * D:2 * GF + (h + 1) * D, :])

        # constant masks / helpers
        tri = pool1.tile([128, 128], BF16)     # tri[p, y] = 1.0 iff y >= p
        nc.gpsimd.memset(tri[:], 1.0)
        nc.gpsimd.affine_select(
            out=tri[:], in_=tri[:],
            compare_op=mybir.AluOpType.is_ge, fill=0.0,
            base=0, pattern=[[1, 128]], channel_multiplier=-1)
        ones1 = pool1.tile([1, D], BF16)
        nc.gpsimd.memset(ones1[:], 1.0)

        # ---- QKV projection --------------------------------------------------
        # q,k transposed: qk_sb[:, ft, t]; ft 0 = q (2 heads), ft 1 = k
        qk_sb = pool1.tile([128, 2, SF], BF16)
        # v natural [token, feat] + ones column: [128, tt, h, 65]
        v_sb = pool1.tile([128, SF // 128, HPC, D + 1], BF16)
        nc.gpsimd.memset(v_sb[:, :, :, D:D + 1], 1.0)

        def project_qk(b):
            # kt-outer, tch-pairs: one LDWEIGHTS per (ft, pair, kt) serves 2 MMs
            for ft in range(2):
                for pr in range(2):
                    tchs = [4 * b + 2 * pr, 4 * b + 2 * pr + 1]
                    ps = [psum_pq.tile([128, 512], F32, tag="pq",
                                       name=f"pqk{ft}{pr}{i}")
                          for i in range(len(tchs))]
                    for kt in range(KT):
                        for i, tch in enumerate(tchs):
                            nc.tensor.matmul(
                                ps[i][:],
                                lhsT=wqkv_bf[:, kt, ft * 128:(ft + 1) * 128],
                                rhs=hst_bf[:, kt, tch * 512:(tch + 1) * 512],
                                start=(kt == 0), stop=(kt == KT - 1),
                            )
                    for i, tch in enumerate(tchs):
                        dst = qk_sb[:, ft, tch * 512:(tch + 1) * 512]
                        if zero_attn_bias:
                            nc.vector.tensor_copy(dst, ps[i][:])
                        else:
                            nc.scalar.activation(
                                dst, ps[i][:],
                                mybir.ActivationFunctionType.Identity,
                                bias=bqk_t[:, ft, :])

        def project_v(b):
            # one accumulation group per PSUM tile: a second group sharing
            # the bank would wipe has_written on its start and drop kt=0
            for tt in range(16 * b, 16 * b + 16):
                ps = psum_pq.tile([128, GF], F32, tag="pq")
                for kt in range(KT):
                    nc.tensor.matmul(
                        ps[:],
                        lhsT=hst_bf[:, kt, tt * 128:(tt + 1) * 128],
                        rhs=wqkv_bf[:, kt, 2 * GF:3 * GF],
                        start=(kt == 0), stop=(kt == KT - 1),
                    )
                nc.vector.tensor_copy(
                    v_sb[:, tt, :, 0:D],
                    ps[:].rearrange("p (h d) -> p h d", h=HPC))

        # ---- attention -------------------------------------------------------
        # normalized attention outputs, head on the FREE axis (all DVE ops at
        # partition base 0); consolidated per-slot into atall by SBUF DMA
        at64 = pool1.tile([D, HPC, NCORES, 512], BF16)
        # c_proj rhs layout: [feat(2 heads stacked on partitions), slot, 512]
        atall = pool1.tile([128, NCORES, 512], BF16)

        pending = []

        def attn_block(b, tb):
            """One 512-query block (both heads, row-packed)."""
            tok0 = b * S
            tt0 = tok0 // 128
            q0 = tok0 + tb * 512
            av = psum_av.tile([D + 1, HPC, 512], F32, tag="av")
            ntj = 4 * (tb + 1)
            for t in range(ntj):
                u = t - 4 * tb              # >=0 only inside diagonal quad
                w = 512 if u < 0 else 512 - 128 * u
                c0 = 512 - w                # query-col offset of this tile
                st = psum_s.tile([128, HPC, 512], F32, tag="st")
                for h in range(HPC):        # row-packed pair: concurrent MMs
                    nc.tensor.matmul(
                        st[:, h, c0:512],
                        lhsT=qk_sb[64 * h:64 * h + 64, 1,
                                   tok0 + t * 128:tok0 + (t + 1) * 128],
                        rhs=qk_sb[64 * h:64 * h + 64, 0, q0 + c0:q0 + 512],
                        start=True, stop=True,
                    )
                if pending and t == 3:
                    pending.pop(0)()
                pt = ppool.tile([128, HPC, 512], BF16, tag="pt")
                nc.scalar.activation(
                    pt[:, :, c0:512], st[:, :, c0:512],
                    mybir.ActivationFunctionType.Exp, scale=0.125)
                if u >= 0:
                    # triangular boundary on the first 128 query cols
                    for h in range(HPC):
                        nc.vector.tensor_mul(
                            pt[:, h, c0:c0 + 128], pt[:, h, c0:c0 + 128],
                            tri[:])
                for h in range(HPC):
                    nc.tensor.matmul(
                        av[:, h, c0:512],
                        lhsT=v_sb[:, tt0 + t, h, :],
                        rhs=pt[:, h, c0:512],
                        start=(t == 0), stop=(t == ntj - 1),
                    )

            # snapshot numerators + denominators out of PSUM at block end so
            # the av slot frees immediately (next block's PV can't race the
            # deferred epilogue)
            avs = ppool.tile([D + 1, HPC, 512], BF16, tag="avs")
            nc.vector.tensor_copy(avs[:], av[:])
            # reciprocal cost scales with free-size per partition: DMA the
            # denominator row across 128 partitions, recip there (~0.2us
            # instead of 6.7us of head-of-line DVE blocking), DMA back
            dent = small.tile([128, 2 * 512 // 128], BF16, tag="dent")
            nc.sync.dma_start(dent[:], avs[D:D + 1, :, :])
            recp = small.tile([128, 2 * 512 // 128], BF16, tag="recp")
            with nc.allow_low_precision("softmax recip bf16 is fine"):
                nc.vector.reciprocal(recp[:], dent[:])
            r2 = small.tile([1, HPC, 512], BF16, tag="r2")
            nc.sync.dma_start(r2[:], recp[:])

            def make_epilogue(avs=avs, r2=r2, slot=4 * b + tb):
                def epi():
                    for h in range(HPC):
                        rb = psum_pq.tile([D, 512], F32, tag="pq",
                                          name=f"rb{h}")
                        nc.tensor.matmul(rb[:], lhsT=ones1[:],
                                         rhs=r2[:, h, :],
                                         start=True, stop=True)
                        dst = at64[:, h, slot, :]
                        if zero_attn_bias:
                            nc.vector.tensor_mul(dst, avs[0:D, h, :], rb[:])
                        else:
                            at = ppool.tile([D, 512], BF16, tag="at")
                            nc.vector.tensor_mul(at[:], avs[0:D, h, :], rb[:])
                            nc.scalar.activation(
                                dst, at[:],
                                mybir.ActivationFunctionType.Identity,
                                bias=bv_t[:, h, :])
                        nc.sync.dma_start(
                            atall[64 * h:64 * h + 64, slot, :], dst)
                return epi
            pending.append(make_epilogue())

        def proj_slots(slots, nrng, mix=False):
            # partial c_proj: out[128n : , 512s : ] = wproj[:, ncols].T @ atall
            for n in nrng:
                for s in slots:
                    ps = psum_pq.tile([128, 512], F32, tag="pq")
                    nc.tensor.matmul(
                        ps[:],
                        lhsT=wproj_bf[:, n * 128:(n + 1) * 128],
                        rhs=atall[:, s, :],
                        start=True, stop=True,
                    )
                    ot = opool.tile([128, 512], BF16, tag="ot")
                    if mix and n % 2 == 1:
                        # tail phase: ACT is idle, split the cast load
                        nc.scalar.activation(
                            ot[:], ps[:],
                            mybir.ActivationFunctionType.Identity)
                    else:
                        nc.vector.tensor_copy(ot[:], ps[:])
                    nc.sync.dma_start(
                        out_ext[n * 128:(n + 1) * 128, s * 512:(s + 1) * 512],
                        ot[:])

        # ---- schedule --------------------------------------------------------
        project_qk(0)
        project_v(0)
        attn_block(0, 3)
        attn_block(0, 2)
        project_qk(1)           # fills PE gaps while attn(b0) is ACT-bound
        attn_block(0, 1)
        project_v(1)
        attn_block(0, 0)
        attn_block(1, 3)
        proj_slots([3, 2], range(8))   # b0 slots stream out during attn(b1)
        attn_block(1, 2)
        proj_slots([1, 0], range(8))
        attn_block(1, 1)
        proj_slots([7], range(8))
        attn_block(1, 0)
        proj_slots([6], range(8))
        while pending:
            pending.pop(0)()
        proj_slots([5, 4], range(8), mix=True)

    nc.finalize()
    return nc


_CACHE = {}


def _get_nc(zero_attn_bias):
    if zero_attn_bias not in _CACHE:
        _CACHE[zero_attn_bias] = build(zero_attn_bias)
    return _CACHE[zero_attn_bias]


def kernel(hidden_states, c_attn_w, c_attn_b, c_proj_w, c_proj_b, **extra):
    hidden_states = np.asarray(hidden_states, np.float32)
    c_attn_w = np.asarray(c_attn_w, np.float32)
    c_attn_b = np.asarray(c_attn_b, np.float32)
    c_proj_w = np.asarray(c_proj_w, np.float32)
    c_proj_b = np.asarray(c_proj_b, np.float32)

    zero_attn_bias = not np.any(c_attn_b)
    nc = _get_nc(zero_attn_bias)

    bf = ml_dtypes.bfloat16
    hsT = np.ascontiguousarray(hidden_states.reshape(B * S, NX).T).astype(bf)

    in_maps = []
    for i in range(NCORES):
        cols = np.r_[i * GF:(i + 1) * GF,
                     NX + i * GF:NX + (i + 1) * GF,
                     2 * NX + i * GF:2 * NX + (i + 1) * GF]
        in_maps.append({
            "hst": hsT,
            "wqkv": np.ascontiguousarray(c_attn_w[:, cols]).astype(bf),
            "bqkv": np.ascontiguousarray(c_attn_b[cols].reshape(3 * GF, 1)),
            "wproj": np.ascontiguousarray(
                c_proj_w[i * GF:(i + 1) * GF, :]).astype(bf),
        })

    res = run_bass_kernel_spmd(nc, in_maps, core_ids=list(range(NCORES)))
    acc = np.zeros((NX, B * S), np.float32)
    for i in range(NCORES):
        acc += np.asarray(res.results[i]["out"]).astype(np.float32)
    out = acc.T + c_proj_b[None, :]
    return np.ascontiguousarray(out.reshape(B, S, NX))


if __name__ == "__main__":
    rng = np.random.default_rng(0)
    hs = rng.standard_normal((B, S, NX), dtype=np.float32)
    wa = (rng.standard_normal((NX, 3 * NX), dtype=np.float32) * 0.02)
    wp = (rng.standard_normal((NX, NX), dtype=np.float32) * 0.02)
    o = kernel(hidden_states=hs, c_attn_w=wa, c_attn_b=np.zeros(3 * NX, np.float32),
               c_proj_w=wp, c_proj_b=np.zeros(NX, np.float32))
    print(o.shape, o.dtype)
